# revision 28
# baseline (speedup 1.0000x reference)
"""EntityAwareAttention TRN2 Bass kernel — 8-core data parallel, v3.

Problem (per full batch): B=64, L=256, H=1024, P=64, A=512, T=8.
  e1_h/e2_h   = word_hiddens gathered at e1_end/e2_end           [B, H]
  e*_type     = softmax(e_h @ tE.T) @ tE                          [B, H]
  ef          = concat(e1_h, e1_type, e2_h, e2_type)              [B, 4H]
  dense_pos   = concat(wh, pos_e1, pos_e2) @ W_pos                [B, L, A]
  dense_ent   = ef @ W_ent                                        [B, A]
  u           = tanh(dense_pos + repeat-interleave(dense_ent))    [B, L, A]
                (addend for (l, a) is dense_ent[b, 2l + (a>=256)])
  vu          = u @ v ; alpha = softmax(vu, axis=L)               [B, L]
  z           = sum_l alpha[b,l] * wh[b,l,:]                      [B, H]

Sharding: batch across 8 cores (8 batches/core); weights replicated.

Design notes:
  * dense_pos k-tiles in mixed precision: N_BF k-tiles bf16 (plain
    matmuls) + the rest fp8e4m3 DoubleRow, each DR pairing the exact
    residual split (W_pos_hi, W_pos_res) of W_pos*32 against a
    stride-0-broadcast whT k-tile. W_pos quantization cancels; only
    whT's e4m3 noise on the covered features remains.
  * dense_ent transposed (deT[a,b], N=8 matmuls, ~free on PE) from
    e3m4 W_ent scaled by 64, repacked into per-partition tanh bias.
  * vu via one fused DVE tensor_tensor_reduce per tile; exp has no
    max-shift (|vu| <= sum|v| ~ 21, safely inside f32/bf16 range) so
    alpha columns are produced per tile and the transposed z matmuls
    (N=1, lhsT = natural bf16 wh tiles) run inside the main loop one
    block behind dense_pos.
  * DMA stream order: W_ent first (its consumer chain is long), then
    W_pos, whT in quarter-token chunks, wh-natural chunks interleaved
    just-in-time for the z matmuls.
"""

import numpy as np
import ml_dtypes

import concourse.bass as bass
import concourse.tile as tile
from concourse import bacc, mybir
from concourse.bass_utils import run_bass_kernel_spmd

F32 = mybir.dt.float32
F32R = mybir.dt.float32r
BF16 = mybir.dt.bfloat16
F8E4 = mybir.dt.float8e4
F8E3 = mybir.dt.float8e3
I32 = mybir.dt.int32
AF = mybir.ActivationFunctionType
ALU = mybir.AluOpType
DR = mybir.MatmulPerfMode.DoubleRow

B, L, H, P2, A, T = 64, 256, 1024, 64, 512, 8
NCORES = 8
BL = B // NCORES            # 8 local batches
TOK = BL * L                # 2048 tokens
NT = TOK // 128             # 16 token tiles
F = H + 2 * P2              # 1152 contraction dim
KF = F // 128               # 9 k-tiles
N_BF = 3                    # k-tiles 0..N_BF-1 in bf16
N_E4 = KF - N_BF            # remaining k-tiles in fp8e4m3 DoubleRow
KE = 4 * H // 128           # 32 W_ent k-tiles
HC = H // 128               # 8 h-chunks
WP_SCALE = 32.0             # W_pos host scale (tanh applies 1/32)
WE_SCALE = 64.0             # W_ent host scale (bias fixup applies 1/64)


def _build_core(tc):
    nc = tc.nc
    whTb_d = nc.dram_tensor("whT_bf", [N_BF * 128, TOK], BF16,
                            kind="ExternalInput").ap()
    whTe_d = nc.dram_tensor("whT_e4", [N_E4 * 128, TOK], F8E4,
                            kind="ExternalInput").ap()
    wposb_d = nc.dram_tensor("wpos_bf", [N_BF * 128, A], BF16,
                             kind="ExternalInput").ap()
    wpose_d = nc.dram_tensor("wpos_e4", [N_E4 * 2 * 128, A], F8E4,
                             kind="ExternalInput").ap()
    went_d = nc.dram_tensor("W_ent", [4 * H, A], F8E3, kind="ExternalInput").ap()
    whn_d = nc.dram_tensor("whn", [TOK, H], BF16, kind="ExternalInput").ap()
    eh_d = nc.dram_tensor("eh", [2 * BL, H], F32R, kind="ExternalInput").ap()
    te_d = nc.dram_tensor("type_embeddings", [T, H], BF16,
                          kind="ExternalInput").ap()
    v_d = nc.dram_tensor("v", [1, A], F32, kind="ExternalInput").ap()
    out_d = nc.dram_tensor("out", [BL, H], F32, kind="ExternalOutput").ap()

    const = tc.alloc_tile_pool(name="const", bufs=1)
    upool = tc.alloc_tile_pool(name="upool", bufs=3)
    spool = tc.alloc_tile_pool(name="spool", bufs=3)
    ps_dp = tc.alloc_tile_pool(name="ps_dp", bufs=4, space="PSUM")
    ps_tr = tc.alloc_tile_pool(name="ps_tr", bufs=2, space="PSUM")
    ps_acc = tc.alloc_tile_pool(name="ps_acc", bufs=1, space="PSUM")

    # ---- small loads ride ACT so the SP queue goes straight to the big
    # streaming DMAs; the entity rows are host-gathered (pure indexing).
    eh = const.tile([2 * BL, H], F32R)
    nc.scalar.dma_start(eh[:], eh_d[:])

    # dummy activation: absorbs the one-time LoadActFuncSet (1.28us)
    # while the DMA stream is still priming, instead of paying it inside
    # the entity-chain critical path at the first real Exp.
    dummy = const.tile([1, 1], F32)
    nc.gpsimd.memset(dummy[:], 0.0)
    nc.scalar.activation(dummy[:], dummy[:], AF.Exp)

    # ---- other constants ----
    iota_p = const.tile([128, 128], I32)
    iota_f = const.tile([128, 128], I32)
    nc.gpsimd.iota(iota_p[:], pattern=[[0, 128]], base=0, channel_multiplier=1)
    nc.gpsimd.iota(iota_f[:], pattern=[[1, 128]], base=0, channel_multiplier=0)
    ident = const.tile([128, 128], F32R)
    nc.vector.tensor_tensor(out=ident[:], in0=iota_p[:], in1=iota_f[:],
                            op=ALU.is_equal)
    ident_bf = const.tile([128, 128], BF16)
    nc.vector.tensor_copy(out=ident_bf[:], in_=ident[:].bitcast(F32))
    ones = const.tile([128, 1], BF16)
    nc.gpsimd.memset(ones[:], 1.0)

    v_sb = const.tile([1, A], F32)
    nc.scalar.dma_start(v_sb[:], v_d[:])
    v_bc = const.tile([128, A], F32)
    nc.gpsimd.partition_broadcast(v_bc[:], v_sb[0:1, :])
    v_bc16 = const.tile([128, A], BF16)
    nc.vector.tensor_copy(out=v_bc16[:], in_=v_bc[:])

    te_sb = const.tile([T, H], BF16)
    nc.scalar.dma_start(te_sb[:], te_d[:])

    # ---- big DMA stream (SP) — order is the DMA-track schedule ----
    wposb = const.tile([128, N_BF * A], BF16)
    wpose = const.tile([128, N_E4 * 2 * A], F8E4)
    whTb = const.tile([128, N_BF * TOK], BF16)
    whTb_cols = whTb_d.rearrange("(k p) t -> p k t", p=128)
    whTe = const.tile([128, N_E4 * TOK], F8E4)
    whTe_cols = whTe_d.rearrange("(k p) t -> p k t", p=128)
    went_sb = const.tile([128, KE * A], F8E3)
    went_rows = went_d.rearrange("(kt p) a -> p kt a", p=128)
    whn_sb = const.tile([128, NT * H], BF16)
    whn_rows = whn_d.rearrange("(i p) h -> p i h", p=128)

    def whT_quarter(q):
        s = slice(q * 512, (q + 1) * 512)
        nc.sync.dma_start(
            whTb.rearrange("p (k t) -> p k t", k=N_BF)[:, :, s],
            whTb_cols[:, :, s])
        nc.sync.dma_start(
            whTe.rearrange("p (k t) -> p k t", k=N_E4)[:, :, s],
            whTe_cols[:, :, s])

    def whn_chunk(j):
        nc.sync.dma_start(
            whn_sb.rearrange("p (i h) -> p i h", i=NT)[:, 4 * j:4 * j + 4],
            whn_rows[:, 4 * j:4 * j + 4])

    nc.sync.dma_start(
        wposb.rearrange("p (k a) -> p k a", k=N_BF),
        wposb_d.rearrange("(k p) a -> p k a", p=128))
    nc.sync.dma_start(
        whTb.rearrange("p (k t) -> p k t", k=N_BF)[:, :, 0:512],
        whTb_cols[:, :, 0:512])
    nc.sync.dma_start(
        wpose.rearrange("p (k w a) -> p k w a", k=N_E4, w=2),
        wpose_d.rearrange("(k w p) a -> p k w a", p=128, w=2))
    nc.sync.dma_start(
        whTe.rearrange("p (k t) -> p k t", k=N_E4)[:, :, 0:512],
        whTe_cols[:, :, 0:512])
    for h in range(2):
        nc.sync.dma_start(
            went_sb.rearrange("p (kt a) -> p kt a", kt=KE)[:, h * 16:(h + 1) * 16],
            went_rows[:, h * 16:(h + 1) * 16])
    whT_quarter(1)
    whT_quarter(2)
    whT_quarter(3)
    for j in range(4):
        whn_chunk(j)

    # ---- type embeddings transposed: teT[:, hc*T:+T] = te[:, hc-chunk].T ----
    # all 8 transposes land in ONE PSUM tile so a single DVE copy drains
    # them (the transpose->copy ping-pong was a 0.7us/hop latency chain).
    teT = const.tile([128, HC * T], BF16)
    ptt = ps_tr.tile([128, 128], F32R, tag="tr")
    pttb = ptt.bitcast(BF16)
    for hc in range(HC):
        nc.tensor.transpose(pttb[:, hc * T:(hc + 1) * T],
                            te_sb[:, hc * 128:(hc + 1) * 128],
                            ident_bf[0:T, 0:T])
    nc.vector.tensor_copy(out=teT[:], in_=pttb[:, 0:HC * T])

    # ---- main loop state ----
    vu = const.tile([128, NT], F32)      # col = half*8 + b
    alph = const.tile([128, NT], BF16)   # exp(vu), same cols
    zT = ps_acc.tile([128, HC * BL], F32, tag="zT")

    def dp_bf(i):
        dp = ps_dp.tile([128, A], F32, tag="dp", name=f"dp{i}")
        for k in range(N_BF):
            nc.tensor.matmul(
                dp[:], lhsT=whTb[:, k * TOK + i * 128: k * TOK + (i + 1) * 128],
                rhs=wposb[:, k * A:(k + 1) * A],
                start=(k == 0), stop=False)
        return dp

    def dp_dr(i, dp):
        for kk in range(N_E4):
            lp = whTe[:, kk * TOK + i * 128: kk * TOK + (i + 1) * 128]
            nc.tensor.matmul(
                dp[:], lhsT=lp.unsqueeze(1).broadcast_to([128, 2, 128]),
                rhs=wpose[:, kk * 2 * A:(kk + 1) * 2 * A].rearrange(
                    "p (w a) -> p w a", w=2),
                start=False, stop=(kk == N_E4 - 1), perf_mode=DR)

    def dp_mms(i):
        dp = dp_bf(i)
        dp_dr(i, dp)
        return dp

    def act_vu(i, dp):
        b, half = i // 2, i % 2
        col = half * 8 + b
        u = upool.tile([128, A], BF16, tag="u")
        nc.scalar.activation(u[:, 0:256], dp[:, 0:256], AF.Tanh,
                             bias=bias_sb[:, half * 8 + b: half * 8 + b + 1],
                             scale=1.0 / WP_SCALE)
        nc.scalar.activation(u[:, 256:512], dp[:, 256:512], AF.Tanh,
                             bias=bias_sb[:, 16 + half * 8 + b: 16 + half * 8 + b + 1],
                             scale=1.0 / WP_SCALE)
        scr = spool.tile([128, A], BF16, tag="scr")
        nc.vector.tensor_tensor_reduce(
            out=scr[:], in0=u[:], in1=v_bc16[:], scale=1.0, scalar=0.0,
            op0=ALU.mult, op1=ALU.add, accum_out=vu[:, col:col + 1])

    def z_mms(i):
        b, half = i // 2, i % 2
        col = half * 8 + b
        for c in range(HC):
            nc.tensor.matmul(
                zT[:, c * BL + b: c * BL + b + 1],
                lhsT=whn_sb[:, i * H + c * 128: i * H + (c + 1) * 128],
                rhs=alph[:, col:col + 1],
                start=(half == 0), stop=(half == 1))

    # first ps_dp-depth dense_pos bf16 parts — PE fills its p-state ramp
    # with real work while the gather/entity chain is still in flight
    dps = {}
    for i in range(4):
        dps[i] = dp_bf(i)

    # ---- entity features efT[:, kt*8:+8], 32 k-tiles ----
    # regions: 0=e1_h(kt0..7) 1=e2_h(8..15) 2=e1_type(16..23) 3=e2_type(24..31)
    # (W_ent rows are host-permuted to match.) All 8 transposes into one
    # PSUM tile, drained by a single reordering DVE copy.
    efT = const.tile([128, KE * BL], BF16)
    pteh = ps_tr.tile([128, 128], F32R, tag="tr")
    for hc in range(HC):
        nc.tensor.transpose(pteh[:, hc * 16:(hc + 1) * 16],
                            eh[:, hc * 128:(hc + 1) * 128],
                            ident[0:2 * BL, 0:2 * BL])
    nc.vector.tensor_copy(
        out=efT[:, 0:128].rearrange("p (two hc b) -> p two hc b", two=2, hc=HC),
        in_=pteh[:].rearrange("p (hc two b) -> p two hc b", hc=HC, two=2)
        .bitcast(F32))

    # type-softmax scores + ACT/DVE stages (PE: just the tiny score mms;
    # the softmax stages run on ACT/DVE under the dense_pos DR matmuls)
    als = []
    for ent in range(2):
        sc = ps_tr.tile([128, 128], F32, tag="tr")
        for hc in range(HC):
            col = (0 if ent == 0 else HC) + hc
            nc.tensor.matmul(sc[0:BL, 0:T], lhsT=efT[:, col * BL:(col + 1) * BL],
                             rhs=teT[:, hc * T:(hc + 1) * T],
                             start=(hc == 0), stop=(hc == HC - 1))
        asm = const.tile([BL, T], F32, tag=f"asm{ent}")
        ssum = const.tile([BL, 1], F32, tag=f"ssum{ent}")
        nc.scalar.activation(asm[:], sc[0:BL, 0:T], AF.Exp, accum_out=ssum[:])
        rs = const.tile([BL, 1], F32, tag=f"rs{ent}")
        nc.vector.reciprocal(rs[:], ssum[:])
        al = const.tile([BL, T], F32R, tag=f"al{ent}")
        nc.vector.tensor_scalar(out=al[:], in0=asm[:], scalar1=rs[:, 0:1],
                                scalar2=None, op0=ALU.mult)
        als.append(al)

    # dense_pos DR parts for tiles 0-3 (data lands after the bf parts)
    for i in range(4):
        dp_dr(i, dps[i])

    # finish entity features: e_type = alpha @ tE into efT regions 2/3
    for ent in range(2):
        pt = ps_tr.tile([128, 128], F32R, tag="tr")
        nc.tensor.transpose(pt[0:T, 0:BL], als[ent][:], ident[0:BL, 0:BL])
        alTe = const.tile([T, BL], BF16, tag=f"alTe{ent}")
        nc.vector.tensor_copy(out=alTe[:], in_=pt[0:T, 0:BL].bitcast(F32))
        ptm = ps_tr.tile([128, 128], F32, tag="tr")
        for hc in range(HC):
            nc.tensor.matmul(ptm[:, hc * BL:(hc + 1) * BL],
                             lhsT=te_sb[:, hc * 128:(hc + 1) * 128],
                             rhs=alTe[:], start=True, stop=True)
        nc.vector.tensor_copy(
            out=efT[:, (2 + ent) * 64:(3 + ent) * 64],
            in_=ptm[:, 0:64])

    # ---- dense_ent transposed: deT[a'(p), b] over 4 permuted a-chunks ----
    # W_ent columns are host-permuted (a' = (par, half, l)) so deT IS the
    # per-partition tanh bias table, up to the 1/WE_SCALE copy below.
    deT = ps_acc.tile([128, 4 * BL], F32, tag="deT")
    for k in range(KE):
        for c in range(4):
            nc.tensor.matmul(
                deT[:, c * BL:(c + 1) * BL],
                lhsT=went_sb[:, k * A + c * 128: k * A + (c + 1) * 128],
                rhs=efT[:, k * BL:(k + 1) * BL],
                start=(k == 0), stop=(k == KE - 1))
    bias_sb = const.tile([128, 32], F32)
    nc.vector.tensor_scalar(out=bias_sb[:], in0=deT[:], scalar1=1.0 / WE_SCALE,
                            scalar2=None, op0=ALU.mult)

    for i in range(4):
        act_vu(i, dps[i])
    for g in range(1, 4):
        for i in range(4 * g, 4 * g + 4):
            dps[i] = dp_mms(i)
        for i in range(4 * g, 4 * g + 4):
            act_vu(i, dps[i])
    # exp of all 16 vu columns in one ACT op (per-tile exps would sit in
    # the ACT queue waiting on the DVE reduce and pace the whole loop)
    nc.scalar.activation(alph[:], vu[:], AF.Exp)

    # z matmuls run after the dp loop (PE is free; wh-natural chunks
    # stream in behind the whT quarters)
    for i in range(0, 12):
        z_mms(i)

    # ---- ers[b] = 1/(esum[b] + esum[8+b]) via two accumulating matmuls ----
    # (PSUM slot reuse: deT was fully consumed by the bias copy long ago)
    esp = ps_acc.tile([128, 4 * BL], F32, tag="deT")
    nc.tensor.matmul(esp[0:BL, 0:1], lhsT=alph[:, 0:BL], rhs=ones[:],
                     start=True, stop=False)
    nc.tensor.matmul(esp[0:BL, 0:1], lhsT=alph[:, BL:NT], rhs=ones[:],
                     start=False, stop=True)
    for i in range(12, NT):
        z_mms(i)
    ers = const.tile([BL, 1], F32)
    nc.vector.reciprocal(ers[:], esp[0:BL, 0:1])

    # ---- epilogue: zT -> SBUF -> transposes into 2 PSUM banks -> 2
    # parallel scaled copies (ACT + DVE) -> out ----
    zsb = const.tile([128, HC * BL], F32R)
    nc.vector.tensor_copy(out=zsb[:].bitcast(F32), in_=zT[:])
    z_sb = const.tile([BL, H], F32)
    zz0 = ps_tr.tile([128, 512], F32R, tag="tr")
    zz1 = ps_tr.tile([128, 512], F32R, tag="tr")
    for c in range(HC):
        zz = zz0 if c < 4 else zz1
        nc.tensor.transpose(zz[0:BL, (c % 4) * 128:(c % 4 + 1) * 128],
                            zsb[:, c * BL:(c + 1) * BL], ident[:, :])
    nc.scalar.activation(z_sb[:, 0:512], zz0[0:BL, :].bitcast(F32), AF.Copy,
                         scale=ers[:, 0:1])
    nc.vector.tensor_scalar(out=z_sb[:, 512:1024], in0=zz1[0:BL, :].bitcast(F32),
                            scalar1=ers[:, 0:1], scalar2=None, op0=ALU.mult)

    nc.sync.dma_start(out_d[:, 0:512], z_sb[:, 0:512])
    nc.sync.dma_start(out_d[:, 512:1024], z_sb[:, 512:1024])

    for p in (ps_acc, ps_tr, ps_dp, spool, upool, const):
        p.release()


def build():
    nc = bacc.Bacc("TRN2", target_bir_lowering=False, debug=False,
                   num_devices=NCORES)
    with tile.TileContext(nc) as tc:
        _build_core(tc)
    nc.compile()
    return nc


_NC = None


def _prep_core(wh, p1, p2, e1, e2, te, wp_bf, wp_e4, we, vv):
    pf = np.empty((TOK, F), dtype=np.float32)
    pf[:, :H] = wh
    pf[:, H:H + P2] = p1
    pf[:, H + P2:] = p2
    pfT = np.ascontiguousarray(pf.T)
    bidx = np.arange(BL)
    eh = np.ascontiguousarray(
        np.concatenate([wh.reshape(BL, L, H)[bidx, e1],
                        wh.reshape(BL, L, H)[bidx, e2]], axis=0))
    return {
        "whT_bf": np.ascontiguousarray(pfT[:N_BF * 128]).astype(ml_dtypes.bfloat16),
        "whT_e4": np.ascontiguousarray(pfT[N_BF * 128:]).astype(ml_dtypes.float8_e4m3),
        "wpos_bf": wp_bf,
        "wpos_e4": wp_e4,
        "W_ent": we,
        "whn": wh.astype(ml_dtypes.bfloat16),
        "eh": eh,
        "type_embeddings": te,
        "v": vv,
    }


def kernel(word_hiddens, pos_e1_embeddings, pos_e2_embeddings, e1_end, e2_end,
           type_embeddings, W_pos, W_ent, v):
    global _NC
    if _NC is None:
        _NC = build()
    wh = np.ascontiguousarray(word_hiddens, dtype=np.float32).reshape(B, L, H)
    p1 = np.ascontiguousarray(pos_e1_embeddings, dtype=np.float32).reshape(B, L, P2)
    p2 = np.ascontiguousarray(pos_e2_embeddings, dtype=np.float32).reshape(B, L, P2)
    e1 = np.asarray(e1_end, dtype=np.int32).reshape(B)
    e2 = np.asarray(e2_end, dtype=np.int32).reshape(B)
    te = np.ascontiguousarray(type_embeddings, dtype=np.float32).astype(ml_dtypes.bfloat16)

    wps = np.ascontiguousarray(W_pos, dtype=np.float32) * WP_SCALE  # [F, A]
    wp_bf = wps[:N_BF * 128].astype(ml_dtypes.bfloat16)
    w_tail = wps[N_BF * 128:].reshape(N_E4, 128, A)
    hi = w_tail.astype(ml_dtypes.float8_e4m3)
    res = (w_tail - hi.astype(np.float32)).astype(ml_dtypes.float8_e4m3)
    wp_e4 = np.ascontiguousarray(
        np.stack([hi, res], axis=1).reshape(N_E4 * 2 * 128, A))

    we0 = np.asarray(W_ent, dtype=np.float32).reshape(4, H, A)
    # rows: efT region order; cols: a' = (par, half, l) -> a = 256*half+2l+par
    # so deT lands directly in tanh-bias layout (col c'*8+b, c'=par*2+half)
    aidx = np.empty(A, np.int64)
    for cpr in range(4):
        half, par = cpr % 2, cpr // 2
        aidx[cpr * 128:(cpr + 1) * 128] = 256 * half + 2 * np.arange(128) + par
    we = np.ascontiguousarray(
        (np.concatenate([we0[0], we0[2], we0[1], we0[3]], axis=0)
         * WE_SCALE)[:, aidx]).astype(ml_dtypes.float8_e3m4)
    vv = np.ascontiguousarray(v, dtype=np.float32).reshape(1, A)

    in_maps = []
    for c in range(NCORES):
        s = slice(c * BL, (c + 1) * BL)
        in_maps.append(_prep_core(
            np.ascontiguousarray(wh[s].reshape(TOK, H)),
            p1[s].reshape(TOK, P2), p2[s].reshape(TOK, P2),
            e1[s], e2[s], te, wp_bf, wp_e4, we, vv))
    res8 = run_bass_kernel_spmd(_NC, in_maps, core_ids=list(range(NCORES)))
    return np.concatenate([res8.results[c]["out"] for c in range(NCORES)], axis=0)


# revision 31
# speedup vs baseline: 1.0017x; 1.0017x over previous
"""EntityAwareAttention TRN2 Bass kernel — 8-core data parallel, v3.

Problem (per full batch): B=64, L=256, H=1024, P=64, A=512, T=8.
  e1_h/e2_h   = word_hiddens gathered at e1_end/e2_end           [B, H]
  e*_type     = softmax(e_h @ tE.T) @ tE                          [B, H]
  ef          = concat(e1_h, e1_type, e2_h, e2_type)              [B, 4H]
  dense_pos   = concat(wh, pos_e1, pos_e2) @ W_pos                [B, L, A]
  dense_ent   = ef @ W_ent                                        [B, A]
  u           = tanh(dense_pos + repeat-interleave(dense_ent))    [B, L, A]
                (addend for (l, a) is dense_ent[b, 2l + (a>=256)])
  vu          = u @ v ; alpha = softmax(vu, axis=L)               [B, L]
  z           = sum_l alpha[b,l] * wh[b,l,:]                      [B, H]

Sharding: batch across 8 cores (8 batches/core); weights replicated.

Design notes:
  * dense_pos k-tiles in mixed precision: N_BF k-tiles bf16 (plain
    matmuls) + the rest fp8e4m3 DoubleRow, each DR pairing the exact
    residual split (W_pos_hi, W_pos_res) of W_pos*32 against a
    stride-0-broadcast whT k-tile. W_pos quantization cancels; only
    whT's e4m3 noise on the covered features remains.
  * dense_ent transposed (deT[a,b], N=8 matmuls, ~free on PE) from
    e3m4 W_ent scaled by 64, repacked into per-partition tanh bias.
  * vu via one fused DVE tensor_tensor_reduce per tile; exp has no
    max-shift (|vu| <= sum|v| ~ 21, safely inside f32/bf16 range) so
    alpha columns are produced per tile and the transposed z matmuls
    (N=1, lhsT = natural bf16 wh tiles) run inside the main loop one
    block behind dense_pos.
  * DMA stream order: W_ent first (its consumer chain is long), then
    W_pos, whT in quarter-token chunks, wh-natural chunks interleaved
    just-in-time for the z matmuls.
"""

import numpy as np
import ml_dtypes

import concourse.bass as bass
import concourse.tile as tile
from concourse import bacc, mybir
from concourse.bass_utils import run_bass_kernel_spmd

F32 = mybir.dt.float32
F32R = mybir.dt.float32r
BF16 = mybir.dt.bfloat16
F8E4 = mybir.dt.float8e4
F8E3 = mybir.dt.float8e3
I32 = mybir.dt.int32
AF = mybir.ActivationFunctionType
ALU = mybir.AluOpType
DR = mybir.MatmulPerfMode.DoubleRow

B, L, H, P2, A, T = 64, 256, 1024, 64, 512, 8
NCORES = 8
BL = B // NCORES            # 8 local batches
TOK = BL * L                # 2048 tokens
NT = TOK // 128             # 16 token tiles
F = H + 2 * P2              # 1152 contraction dim
KF = F // 128               # 9 k-tiles
N_BF = 3                    # k-tiles 0..N_BF-1 in bf16
N_E4 = KF - N_BF            # remaining k-tiles in fp8e4m3 DoubleRow
KE = 4 * H // 128           # 32 W_ent k-tiles
HC = H // 128               # 8 h-chunks
WP_SCALE = 32.0             # W_pos host scale (tanh applies 1/32)
WE_SCALE = 64.0             # W_ent host scale (bias fixup applies 1/64)


def _build_core(tc):
    nc = tc.nc
    whTb_d = nc.dram_tensor("whT_bf", [N_BF * 128, TOK], BF16,
                            kind="ExternalInput").ap()
    whTe_d = nc.dram_tensor("whT_e4", [N_E4 * 128, TOK], F8E4,
                            kind="ExternalInput").ap()
    wposb_d = nc.dram_tensor("wpos_bf", [N_BF * 128, A], BF16,
                             kind="ExternalInput").ap()
    wpose_d = nc.dram_tensor("wpos_e4", [N_E4 * 2 * 128, A], F8E4,
                             kind="ExternalInput").ap()
    went_d = nc.dram_tensor("W_ent", [4 * H, A], F8E3, kind="ExternalInput").ap()
    whn_d = nc.dram_tensor("whn", [TOK, H], BF16, kind="ExternalInput").ap()
    eh_d = nc.dram_tensor("eh", [2 * BL, H], F32R, kind="ExternalInput").ap()
    te_d = nc.dram_tensor("type_embeddings", [T, H], BF16,
                          kind="ExternalInput").ap()
    v_d = nc.dram_tensor("v", [1, A], F32, kind="ExternalInput").ap()
    out_d = nc.dram_tensor("out", [BL, H], F32, kind="ExternalOutput").ap()

    const = tc.alloc_tile_pool(name="const", bufs=1)
    upool = tc.alloc_tile_pool(name="upool", bufs=3)
    spool = tc.alloc_tile_pool(name="spool", bufs=3)
    ps_dp = tc.alloc_tile_pool(name="ps_dp", bufs=4, space="PSUM")
    ps_tr = tc.alloc_tile_pool(name="ps_tr", bufs=2, space="PSUM")
    ps_acc = tc.alloc_tile_pool(name="ps_acc", bufs=1, space="PSUM")

    # ---- entity rows (host-gathered, pure indexing) + type embeddings
    # lead the SP stream: their consumer chain is long and HWDGE would
    # park ACT-issued DMAs behind every SP transfer.
    eh = const.tile([2 * BL, H], F32R)
    nc.sync.dma_start(eh[:], eh_d[:])
    te_sb = const.tile([T, H], BF16)
    nc.sync.dma_start(te_sb[:], te_d[:])

    # dummy activation: absorbs the one-time LoadActFuncSet (1.28us)
    # while the DMA stream is still priming, instead of paying it inside
    # the entity-chain critical path at the first real Exp.
    dummy = const.tile([1, 1], F32)
    nc.gpsimd.memset(dummy[:], 0.0)
    nc.scalar.activation(dummy[:], dummy[:], AF.Exp)

    # ---- other constants ----
    iota_p = const.tile([128, 128], I32)
    iota_f = const.tile([128, 128], I32)
    nc.gpsimd.iota(iota_p[:], pattern=[[0, 128]], base=0, channel_multiplier=1)
    nc.gpsimd.iota(iota_f[:], pattern=[[1, 128]], base=0, channel_multiplier=0)
    ident = const.tile([128, 128], F32R)
    nc.vector.tensor_tensor(out=ident[:], in0=iota_p[:], in1=iota_f[:],
                            op=ALU.is_equal)
    ident_bf = const.tile([128, 128], BF16)
    nc.vector.tensor_copy(out=ident_bf[:], in_=ident[:].bitcast(F32))
    ones = const.tile([128, 1], BF16)
    nc.gpsimd.memset(ones[:], 1.0)

    v_sb = const.tile([1, A], F32)
    nc.scalar.dma_start(v_sb[:], v_d[:])
    v_bc = const.tile([128, A], F32)
    nc.gpsimd.partition_broadcast(v_bc[:], v_sb[0:1, :])
    v_bc16 = const.tile([128, A], BF16)
    nc.vector.tensor_copy(out=v_bc16[:], in_=v_bc[:])

    # ---- big DMA stream (SP) — order is the DMA-track schedule ----
    wposb = const.tile([128, N_BF * A], BF16)
    wpose = const.tile([128, N_E4 * 2 * A], F8E4)
    whTb = const.tile([128, N_BF * TOK], BF16)
    whTb_cols = whTb_d.rearrange("(k p) t -> p k t", p=128)
    whTe = const.tile([128, N_E4 * TOK], F8E4)
    whTe_cols = whTe_d.rearrange("(k p) t -> p k t", p=128)
    went_sb = const.tile([128, KE * A], F8E3)
    went_rows = went_d.rearrange("(kt p) a -> p kt a", p=128)
    whn_sb = const.tile([128, NT * H], BF16)
    whn_rows = whn_d.rearrange("(i p) h -> p i h", p=128)

    def whT_quarter(q):
        s = slice(q * 512, (q + 1) * 512)
        nc.sync.dma_start(
            whTb.rearrange("p (k t) -> p k t", k=N_BF)[:, :, s],
            whTb_cols[:, :, s])
        nc.sync.dma_start(
            whTe.rearrange("p (k t) -> p k t", k=N_E4)[:, :, s],
            whTe_cols[:, :, s])

    def whn_chunk(j):
        nc.sync.dma_start(
            whn_sb.rearrange("p (i h) -> p i h", i=NT)[:, 4 * j:4 * j + 4],
            whn_rows[:, 4 * j:4 * j + 4])

    nc.sync.dma_start(
        wposb.rearrange("p (k a) -> p k a", k=N_BF),
        wposb_d.rearrange("(k p) a -> p k a", p=128))
    nc.sync.dma_start(
        whTb.rearrange("p (k t) -> p k t", k=N_BF)[:, :, 0:512],
        whTb_cols[:, :, 0:512])
    nc.sync.dma_start(
        wpose.rearrange("p (k w a) -> p k w a", k=N_E4, w=2),
        wpose_d.rearrange("(k w p) a -> p k w a", p=128, w=2))
    nc.sync.dma_start(
        whTe.rearrange("p (k t) -> p k t", k=N_E4)[:, :, 0:512],
        whTe_cols[:, :, 0:512])
    for h in range(2):
        nc.sync.dma_start(
            went_sb.rearrange("p (kt a) -> p kt a", kt=KE)[:, h * 16:(h + 1) * 16],
            went_rows[:, h * 16:(h + 1) * 16])
    whT_quarter(1)
    whT_quarter(2)
    whT_quarter(3)
    for j in range(4):
        whn_chunk(j)

    # ---- type embeddings transposed: teT[:, hc*T:+T] = te[:, hc-chunk].T ----
    # all 8 transposes land in ONE PSUM tile so a single DVE copy drains
    # them (the transpose->copy ping-pong was a 0.7us/hop latency chain).
    teT = const.tile([128, HC * T], BF16)
    ptt = ps_tr.tile([128, 128], F32R, tag="tr")
    pttb = ptt.bitcast(BF16)
    for hc in range(HC):
        nc.tensor.transpose(pttb[:, hc * T:(hc + 1) * T],
                            te_sb[:, hc * 128:(hc + 1) * 128],
                            ident_bf[0:T, 0:T])
    nc.vector.tensor_copy(out=teT[:], in_=pttb[:, 0:HC * T])

    # ---- main loop state ----
    vu = const.tile([128, NT], F32)      # col = half*8 + b
    alph = const.tile([128, NT], BF16)   # exp(vu), same cols
    zT = ps_acc.tile([128, HC * BL], F32, tag="zT")

    def dp_bf(i):
        dp = ps_dp.tile([128, A], F32, tag="dp", name=f"dp{i}")
        for k in range(N_BF):
            nc.tensor.matmul(
                dp[:], lhsT=whTb[:, k * TOK + i * 128: k * TOK + (i + 1) * 128],
                rhs=wposb[:, k * A:(k + 1) * A],
                start=(k == 0), stop=False)
        return dp

    def dp_dr(i, dp):
        for kk in range(N_E4):
            lp = whTe[:, kk * TOK + i * 128: kk * TOK + (i + 1) * 128]
            nc.tensor.matmul(
                dp[:], lhsT=lp.unsqueeze(1).broadcast_to([128, 2, 128]),
                rhs=wpose[:, kk * 2 * A:(kk + 1) * 2 * A].rearrange(
                    "p (w a) -> p w a", w=2),
                start=False, stop=(kk == N_E4 - 1), perf_mode=DR)

    def dp_mms(i):
        dp = dp_bf(i)
        dp_dr(i, dp)
        return dp

    def act_vu(i, dp):
        b, half = i // 2, i % 2
        col = half * 8 + b
        u = upool.tile([128, A], BF16, tag="u")
        nc.scalar.activation(u[:, 0:256], dp[:, 0:256], AF.Tanh,
                             bias=bias_sb[:, half * 8 + b: half * 8 + b + 1],
                             scale=1.0 / WP_SCALE)
        nc.scalar.activation(u[:, 256:512], dp[:, 256:512], AF.Tanh,
                             bias=bias_sb[:, 16 + half * 8 + b: 16 + half * 8 + b + 1],
                             scale=1.0 / WP_SCALE)
        scr = spool.tile([128, A], BF16, tag="scr")
        nc.vector.tensor_tensor_reduce(
            out=scr[:], in0=u[:], in1=v_bc16[:], scale=1.0, scalar=0.0,
            op0=ALU.mult, op1=ALU.add, accum_out=vu[:, col:col + 1])

    def z_mms(i):
        b, half = i // 2, i % 2
        col = half * 8 + b
        for c in range(HC):
            nc.tensor.matmul(
                zT[:, c * BL + b: c * BL + b + 1],
                lhsT=whn_sb[:, i * H + c * 128: i * H + (c + 1) * 128],
                rhs=alph[:, col:col + 1],
                start=(half == 0), stop=(half == 1))

    # first ps_dp-depth dense_pos bf16 parts — PE fills its p-state ramp
    # with real work while the gather/entity chain is still in flight
    dps = {}
    for i in range(4):
        dps[i] = dp_bf(i)

    # ---- entity features efT[:, kt*8:+8], 32 k-tiles ----
    # regions: 0=e1_h(kt0..7) 1=e2_h(8..15) 2=e1_type(16..23) 3=e2_type(24..31)
    # (W_ent rows are host-permuted to match.) All 8 transposes into one
    # PSUM tile, drained by a single reordering DVE copy.
    efT = const.tile([128, KE * BL], BF16)
    pteh = ps_tr.tile([128, 128], F32R, tag="tr")
    for hc in range(HC):
        nc.tensor.transpose(pteh[:, hc * 16:(hc + 1) * 16],
                            eh[:, hc * 128:(hc + 1) * 128],
                            ident[0:2 * BL, 0:2 * BL])
    nc.vector.tensor_copy(
        out=efT[:, 0:128].rearrange("p (two hc b) -> p two hc b", two=2, hc=HC),
        in_=pteh[:].rearrange("p (hc two b) -> p two hc b", hc=HC, two=2)
        .bitcast(F32))

    # type-softmax scores + ACT/DVE stages (PE: just the tiny score mms;
    # the softmax stages run on ACT/DVE under the dense_pos DR matmuls)
    als = []
    for ent in range(2):
        sc = ps_tr.tile([128, 128], F32, tag="tr")
        for hc in range(HC):
            col = (0 if ent == 0 else HC) + hc
            nc.tensor.matmul(sc[0:BL, 0:T], lhsT=efT[:, col * BL:(col + 1) * BL],
                             rhs=teT[:, hc * T:(hc + 1) * T],
                             start=(hc == 0), stop=(hc == HC - 1))
        asm = const.tile([BL, T], F32, tag=f"asm{ent}")
        ssum = const.tile([BL, 1], F32, tag=f"ssum{ent}")
        nc.scalar.activation(asm[:], sc[0:BL, 0:T], AF.Exp, accum_out=ssum[:])
        rs = const.tile([BL, 1], F32, tag=f"rs{ent}")
        nc.vector.reciprocal(rs[:], ssum[:])
        al = const.tile([BL, T], F32R, tag=f"al{ent}")
        nc.vector.tensor_scalar(out=al[:], in0=asm[:], scalar1=rs[:, 0:1],
                                scalar2=None, op0=ALU.mult)
        als.append(al)

    # finish entity features: e_type = alpha @ tE into efT regions 2/3
    # (before the dp DR parts on the PE queue — the bias chain is longer)
    for ent in range(2):
        pt = ps_tr.tile([128, 128], F32R, tag="tr")
        nc.tensor.transpose(pt[0:T, 0:BL], als[ent][:], ident[0:BL, 0:BL])
        alTe = const.tile([T, BL], BF16, tag=f"alTe{ent}")
        nc.vector.tensor_copy(out=alTe[:], in_=pt[0:T, 0:BL].bitcast(F32))
        ptm = ps_tr.tile([128, 128], F32, tag="tr")
        for hc in range(HC):
            nc.tensor.matmul(ptm[:, hc * BL:(hc + 1) * BL],
                             lhsT=te_sb[:, hc * 128:(hc + 1) * 128],
                             rhs=alTe[:], start=True, stop=True)
        nc.vector.tensor_copy(
            out=efT[:, (2 + ent) * 64:(3 + ent) * 64],
            in_=ptm[:, 0:64])

    # dense_pos DR parts for tiles 0-3 (data lands after the bf parts)
    for i in range(4):
        dp_dr(i, dps[i])

    # ---- dense_ent transposed: deT[a'(p), b] over 4 permuted a-chunks ----
    # W_ent columns are host-permuted (a' = (par, half, l)) so deT IS the
    # per-partition tanh bias table, up to the 1/WE_SCALE copy below.
    deT = ps_acc.tile([128, 4 * BL], F32, tag="deT")
    for k in range(KE):
        for c in range(4):
            nc.tensor.matmul(
                deT[:, c * BL:(c + 1) * BL],
                lhsT=went_sb[:, k * A + c * 128: k * A + (c + 1) * 128],
                rhs=efT[:, k * BL:(k + 1) * BL],
                start=(k == 0), stop=(k == KE - 1))
    bias_sb = const.tile([128, 32], F32)
    nc.vector.tensor_scalar(out=bias_sb[:], in0=deT[:], scalar1=1.0 / WE_SCALE,
                            scalar2=None, op0=ALU.mult)

    for i in range(4):
        act_vu(i, dps[i])
    for g in range(1, 4):
        for i in range(4 * g, 4 * g + 4):
            dps[i] = dp_mms(i)
        for i in range(4 * g, 4 * g + 4):
            act_vu(i, dps[i])
    # exp of all 16 vu columns in one ACT op (per-tile exps would sit in
    # the ACT queue waiting on the DVE reduce and pace the whole loop)
    nc.scalar.activation(alph[:], vu[:], AF.Exp)

    # z matmuls run after the dp loop (PE is free; wh-natural chunks
    # stream in behind the whT quarters)
    for i in range(0, 12):
        z_mms(i)

    # ---- ers[b] = 1/(esum[b] + esum[8+b]) via two accumulating matmuls ----
    # (PSUM slot reuse: deT was fully consumed by the bias copy long ago)
    esp = ps_acc.tile([128, 4 * BL], F32, tag="deT")
    nc.tensor.matmul(esp[0:BL, 0:1], lhsT=alph[:, 0:BL], rhs=ones[:],
                     start=True, stop=False)
    nc.tensor.matmul(esp[0:BL, 0:1], lhsT=alph[:, BL:NT], rhs=ones[:],
                     start=False, stop=True)
    for i in range(12, NT):
        z_mms(i)
    ers = const.tile([BL, 1], F32)
    nc.vector.reciprocal(ers[:], esp[0:BL, 0:1])

    # ---- epilogue: zT -> SBUF -> transposes into 2 PSUM banks -> 2
    # parallel scaled copies (ACT + DVE) -> out ----
    zsb = const.tile([128, HC * BL], F32R)
    nc.vector.tensor_copy(out=zsb[:].bitcast(F32), in_=zT[:])
    z_sb = const.tile([BL, H], F32)
    zz0 = ps_tr.tile([128, 512], F32R, tag="tr")
    zz1 = ps_tr.tile([128, 512], F32R, tag="tr")
    for c in range(HC):
        zz = zz0 if c < 4 else zz1
        nc.tensor.transpose(zz[0:BL, (c % 4) * 128:(c % 4 + 1) * 128],
                            zsb[:, c * BL:(c + 1) * BL], ident[:, :])
    nc.scalar.activation(z_sb[:, 0:512], zz0[0:BL, :].bitcast(F32), AF.Copy,
                         scale=ers[:, 0:1])
    nc.vector.tensor_scalar(out=z_sb[:, 512:1024], in0=zz1[0:BL, :].bitcast(F32),
                            scalar1=ers[:, 0:1], scalar2=None, op0=ALU.mult)

    nc.sync.dma_start(out_d[:, 0:512], z_sb[:, 0:512])
    nc.sync.dma_start(out_d[:, 512:1024], z_sb[:, 512:1024])

    for p in (ps_acc, ps_tr, ps_dp, spool, upool, const):
        p.release()


def build():
    nc = bacc.Bacc("TRN2", target_bir_lowering=False, debug=False,
                   num_devices=NCORES)
    with tile.TileContext(nc) as tc:
        _build_core(tc)
    nc.compile()
    return nc


_NC = None


def _prep_core(wh, p1, p2, e1, e2, te, wp_bf, wp_e4, we, vv):
    pf = np.empty((TOK, F), dtype=np.float32)
    pf[:, :H] = wh
    pf[:, H:H + P2] = p1
    pf[:, H + P2:] = p2
    pfT = np.ascontiguousarray(pf.T)
    bidx = np.arange(BL)
    eh = np.ascontiguousarray(
        np.concatenate([wh.reshape(BL, L, H)[bidx, e1],
                        wh.reshape(BL, L, H)[bidx, e2]], axis=0))
    return {
        "whT_bf": np.ascontiguousarray(pfT[:N_BF * 128]).astype(ml_dtypes.bfloat16),
        "whT_e4": np.ascontiguousarray(pfT[N_BF * 128:]).astype(ml_dtypes.float8_e4m3),
        "wpos_bf": wp_bf,
        "wpos_e4": wp_e4,
        "W_ent": we,
        "whn": wh.astype(ml_dtypes.bfloat16),
        "eh": eh,
        "type_embeddings": te,
        "v": vv,
    }


def kernel(word_hiddens, pos_e1_embeddings, pos_e2_embeddings, e1_end, e2_end,
           type_embeddings, W_pos, W_ent, v):
    global _NC
    if _NC is None:
        _NC = build()
    wh = np.ascontiguousarray(word_hiddens, dtype=np.float32).reshape(B, L, H)
    p1 = np.ascontiguousarray(pos_e1_embeddings, dtype=np.float32).reshape(B, L, P2)
    p2 = np.ascontiguousarray(pos_e2_embeddings, dtype=np.float32).reshape(B, L, P2)
    e1 = np.asarray(e1_end, dtype=np.int32).reshape(B)
    e2 = np.asarray(e2_end, dtype=np.int32).reshape(B)
    te = np.ascontiguousarray(type_embeddings, dtype=np.float32).astype(ml_dtypes.bfloat16)

    wps = np.ascontiguousarray(W_pos, dtype=np.float32) * WP_SCALE  # [F, A]
    wp_bf = wps[:N_BF * 128].astype(ml_dtypes.bfloat16)
    w_tail = wps[N_BF * 128:].reshape(N_E4, 128, A)
    hi = w_tail.astype(ml_dtypes.float8_e4m3)
    res = (w_tail - hi.astype(np.float32)).astype(ml_dtypes.float8_e4m3)
    wp_e4 = np.ascontiguousarray(
        np.stack([hi, res], axis=1).reshape(N_E4 * 2 * 128, A))

    we0 = np.asarray(W_ent, dtype=np.float32).reshape(4, H, A)
    # rows: efT region order; cols: a' = (par, half, l) -> a = 256*half+2l+par
    # so deT lands directly in tanh-bias layout (col c'*8+b, c'=par*2+half)
    aidx = np.empty(A, np.int64)
    for cpr in range(4):
        half, par = cpr % 2, cpr // 2
        aidx[cpr * 128:(cpr + 1) * 128] = 256 * half + 2 * np.arange(128) + par
    we = np.ascontiguousarray(
        (np.concatenate([we0[0], we0[2], we0[1], we0[3]], axis=0)
         * WE_SCALE)[:, aidx]).astype(ml_dtypes.float8_e3m4)
    vv = np.ascontiguousarray(v, dtype=np.float32).reshape(1, A)

    in_maps = []
    for c in range(NCORES):
        s = slice(c * BL, (c + 1) * BL)
        in_maps.append(_prep_core(
            np.ascontiguousarray(wh[s].reshape(TOK, H)),
            p1[s].reshape(TOK, P2), p2[s].reshape(TOK, P2),
            e1[s], e2[s], te, wp_bf, wp_e4, we, vv))
    res8 = run_bass_kernel_spmd(_NC, in_maps, core_ids=list(range(NCORES)))
    return np.concatenate([res8.results[c]["out"] for c in range(NCORES)], axis=0)
